# revision 1
# baseline (speedup 1.0000x reference)
"""Fused EmbeddingBag(mean) + Linear kernel for Trainium2, 8-core data-parallel.

Strategy: batch is sharded 8 ways (2048 bags/core). The embedding table gets a
host-appended zero row; invalid (beyond-length) token slots are redirected to it
on-device, so the length-masked sum becomes a plain sum. Per 128-bag tile, one
indirect DMA gathers all 6400 token rows (each partition = one bag's 50
embeddings), a strided-AP vector reduce sums over the 50 slots, and a single
matmul against [W.T; b; null_emb] applies projection, bias, and the
empty-bag null-embedding select in one shot.
"""

import sys

sys.path.insert(0, "/opt/trn_rl_repo")

from contextlib import ExitStack

import numpy as np

import concourse.bass as bass
import concourse.bacc as bacc
import concourse.mybir as mybir
import concourse.tile as tile
from concourse.bass import IndirectOffsetOnAxis
from concourse.masks import make_identity

VOCAB, EMBED, COND = 100000, 64, 256
B, L = 16384, 50
NCORES = 8
BLOC = B // NCORES  # 2048 bags per core
P = 128
NT = BLOC // P  # 16 tiles per core

F32 = mybir.dt.float32
I32 = mybir.dt.int32


def build_nc(g_bufs: int = 3) -> bass.Bass:
    nc = bacc.Bacc("TRN2", target_bir_lowering=False)

    ids = nc.declare_dram_parameter("ids", [BLOC, L + 1], I32, isOutput=False)
    emb = nc.declare_dram_parameter("emb", [VOCAB + 1, EMBED], F32, isOutput=False)
    wext = nc.declare_dram_parameter("wext", [EMBED + 2, COND], F32, isOutput=False)
    out = nc.declare_dram_parameter("out", [BLOC, COND], F32, isOutput=True)

    op = mybir.AluOpType

    with tile.TileContext(nc) as tc, ExitStack() as ctx:
        const = ctx.enter_context(tc.tile_pool(name="const", bufs=1))
        sb = ctx.enter_context(tc.tile_pool(name="sb", bufs=6))
        gp = ctx.enter_context(tc.tile_pool(name="gp", bufs=g_bufs))
        ps = ctx.enter_context(tc.tile_pool(name="ps", bufs=2, space="PSUM"))

        # One-time constants
        idt = const.tile([P, P], F32, tag="idt")
        make_identity(nc, idt[:])
        iota_l = const.tile([P, L], I32, tag="iota")
        nc.gpsimd.iota(out=iota_l[:], pattern=[[1, L]], base=0, channel_multiplier=0)
        bigc = const.tile([P, L], I32, tag="bigc")
        nc.gpsimd.memset(bigc[:], VOCAB)  # index of the all-zero row
        wext_sb = const.tile([EMBED + 2, COND], F32, tag="wext")
        nc.gpsimd.dma_start(out=wext_sb[:], in_=wext[:])

        for t in range(NT):
            rows = slice(t * P, (t + 1) * P)

            ids_t = sb.tile([P, L + 1], I32, tag="ids")
            nc.gpsimd.dma_start(out=ids_t[:], in_=ids[rows, :])

            lenf = sb.tile([P, 1], F32, tag="lenf")
            nc.vector.tensor_copy(out=lenf[:], in_=ids_t[:, L : L + 1])

            # mask[p, l] = l < len[p]; invalid slots -> zero-row index
            mask_t = sb.tile([P, L], I32, tag="mask")
            nc.vector.tensor_scalar(
                out=mask_t[:], in0=iota_l[:], scalar1=lenf[:, :1], scalar2=None,
                op0=op.is_lt,
            )
            idx_t = sb.tile([P, L], I32, tag="idx")
            nc.vector.select(
                out=idx_t[:], mask=mask_t[:], on_true=ids_t[:, 0:L], on_false=bigc[:]
            )

            # Gather all 50 embeddings per bag: partition p gets bag t*128+p.
            g_t = gp.tile([P, L * EMBED], F32, tag="g")
            for l in range(L):
                nc.gpsimd.indirect_dma_start(
                    out=g_t[:, l * EMBED : (l + 1) * EMBED],
                    out_offset=None,
                    in_=emb[:],
                    in_offset=IndirectOffsetOnAxis(ap=idx_t[:, l : l + 1], axis=0),
                )

            # Sum over the 50 slots (strided view [P, e, l], reduce innermost l)
            s_t = sb.tile([P, EMBED], F32, tag="s")
            nc.vector.tensor_reduce(
                out=s_t[:],
                in_=g_t[:].rearrange("p (l e) -> p e l", l=L, e=EMBED),
                axis=mybir.AxisListType.X,
                op=op.add,
            )

            # mean = sum / max(len, 1); flags for bias-vs-null selection
            den = sb.tile([P, 1], F32, tag="den")
            nc.vector.tensor_scalar_max(out=den[:], in0=lenf[:], scalar1=1.0)
            rec = sb.tile([P, 1], F32, tag="rec")
            nc.vector.reciprocal(out=rec[:], in_=den[:])

            tr = sb.tile([P, EMBED + 2], F32, tag="tr")
            nc.vector.tensor_scalar_mul(
                out=tr[:, 0:EMBED], in0=s_t[:], scalar1=rec[:, :1]
            )
            nc.vector.tensor_scalar(
                out=tr[:, EMBED : EMBED + 1], in0=lenf[:], scalar1=0.0, scalar2=None,
                op0=op.is_gt,
            )
            nc.vector.tensor_scalar(
                out=tr[:, EMBED + 1 : EMBED + 2], in0=lenf[:], scalar1=0.0,
                scalar2=None, op0=op.is_le,
            )

            # [P, 66] -> [66, P] so the projection contracts over E on partitions
            pT = ps.tile([EMBED + 2, P], F32, tag="pT", space="PSUM")
            nc.tensor.transpose(out=pT[:], in_=tr[:], identity=idt[:])
            mT = sb.tile([EMBED + 2, P], F32, tag="mT")
            nc.scalar.copy(out=mT[:], in_=pT[:])

            # out[128, 256] = meanT.T @ [W.T; b; null]: proj + bias + null select
            po = ps.tile([P, COND], F32, tag="po", space="PSUM")
            nc.tensor.matmul(out=po[:], lhsT=mT[:], rhs=wext_sb[:], start=True, stop=True)
            ob = sb.tile([P, COND], F32, tag="ob")
            nc.scalar.copy(out=ob[:], in_=po[:])
            nc.gpsimd.dma_start(out=out[rows, :], in_=ob[:])

    nc.compile()
    return nc


_NC_CACHE: dict = {}


def _get_nc(g_bufs: int = 3) -> bass.Bass:
    if g_bufs not in _NC_CACHE:
        _NC_CACHE[g_bufs] = build_nc(g_bufs)
    return _NC_CACHE[g_bufs]


def make_in_maps(token_ids, lengths, emb_table, W, b, null_emb):
    lens32 = np.asarray(lengths).astype(np.int32, copy=False).reshape(B, 1)
    ids32 = np.ascontiguousarray(
        np.concatenate(
            [np.asarray(token_ids).astype(np.int32, copy=False), lens32], axis=1
        )
    )
    emb_ext = np.concatenate(
        [np.asarray(emb_table, dtype=np.float32), np.zeros((1, EMBED), np.float32)]
    )
    wext = np.concatenate(
        [
            np.asarray(W, dtype=np.float32).T,  # [64, 256]
            np.asarray(b, dtype=np.float32)[None, :],
            np.asarray(null_emb, dtype=np.float32)[None, :],
        ]
    )  # [66, 256]
    return [
        {
            "ids": ids32[c * BLOC : (c + 1) * BLOC],
            "emb": emb_ext,
            "wext": wext,
        }
        for c in range(NCORES)
    ]


def kernel(token_ids, lengths, emb_table, W, b, null_emb, **run_kwargs):
    from concourse.bass_utils import run_bass_kernel_spmd

    nc = _get_nc()
    in_maps = make_in_maps(token_ids, lengths, emb_table, W, b, null_emb)
    res = run_bass_kernel_spmd(nc, in_maps, core_ids=list(range(NCORES)), **run_kwargs)
    out = np.concatenate([res.results[c]["out"] for c in range(NCORES)], axis=0)
    return out



# revision 7
# speedup vs baseline: 11.6362x; 11.6362x over previous
"""Fused EmbeddingBag(mean) + Linear kernel for Trainium2, 8-core data-parallel.

Strategy: batch is sharded 8 ways (2048 bags/core). Each core processes 16
tiles of 128 bags. The gather uses the dedicated SWDGE dma_gather
(InstDMAGatherAnt) instruction — one instruction gathers thousands of rows, vs
~1us fixed overhead per indirect_dma_start that can only fetch 128 rows.

dma_gather indices are int16, so they can only address a 32768-row window of
the table. The table is therefore rebuilt on the host with a zero row
interleaved every 32767 vocab rows (new_id = id + id//32767 + 1), giving four
windows of 32768 rows whose first row is all-zero. Per 128-bag tile and per
window, the bag's tokens belonging to that window are packed into columns
(bag = partition = stream position % 128), padded with window-relative index 0
(the zero row), so invalid/missing slots gather zeros and the plain column sum
equals the masked sum. Bags are sorted by length per core so tiles have
homogeneous lengths and the per-tile column budgets (computed from the actual
data at build time, maxed across cores so all cores share one program) stay
small.

A strided-AP vector reduce sums each bag's columns, and a single matmul
against [W.T; b; null_embedding] applies projection, bias, and the empty-bag
null-embedding select in one shot (per-bag scale 1/max(len,1) and the two
select flags are host-precomputed). Regular DMAs ride the HWDGE queues on the
sync/scalar engines so gpsimd only issues gathers. The host un-permutes the
sorted outputs at the end.
"""

import sys

sys.path.insert(0, "/opt/trn_rl_repo")

from contextlib import ExitStack

import numpy as np

import concourse.bass as bass
import concourse.bacc as bacc
import concourse.mybir as mybir
import concourse.tile as tile
from concourse.masks import make_identity

VOCAB, EMBED, COND = 100000, 64, 256
B, L = 16384, 50
NCORES = 8
BLOC = B // NCORES  # 2048 bags per core
P = 128
NT = BLOC // P  # 16 tiles per core
NWIN = 4
WROWS = 32768  # rows per index window
VROWS = WROWS - 1  # vocab rows per window (row 0 of each window is zero)
TROWS = VOCAB + NWIN  # remapped table rows

F32 = mybir.dt.float32
I32 = mybir.dt.int32
I16 = mybir.dt.int16
KCHUNK = 8  # max gather columns per dma_gather (1024-index SWDGE ring limit)


def build_nc(kmat) -> bass.Bass:
    """kmat[t][w] = column budget for tile t, window w (same for all cores)."""
    ktot = [sum(kr) for kr in kmat]
    idx_cols = [8 * kt for kt in ktot]  # wrapped idx width per tile (int16)
    idx_off = np.concatenate([[0], np.cumsum(idx_cols)]).tolist()

    nc = bacc.Bacc("TRN2", target_bir_lowering=False)

    idx = nc.declare_dram_parameter("idx", [P, idx_off[-1]], I16, isOutput=False)
    aux = nc.declare_dram_parameter("aux", [BLOC, 3], F32, isOutput=False)
    emb = nc.declare_dram_parameter("emb", [TROWS, EMBED], F32, isOutput=False)
    wext = nc.declare_dram_parameter("wext", [EMBED + 2, COND], F32, isOutput=False)
    out = nc.declare_dram_parameter("out", [BLOC, COND], F32, isOutput=True)

    op = mybir.AluOpType

    with tile.TileContext(nc) as tc, ExitStack() as ctx:
        const = ctx.enter_context(tc.tile_pool(name="const", bufs=1))
        sb = ctx.enter_context(tc.tile_pool(name="sb", bufs=4))
        gp = ctx.enter_context(tc.tile_pool(name="gp", bufs=3))
        ps = ctx.enter_context(tc.tile_pool(name="ps", bufs=2, space="PSUM"))

        # One-time constants
        idt = const.tile([P, P], F32, tag="idt")
        make_identity(nc, idt[:])
        wext_sb = const.tile([EMBED + 2, COND], F32, tag="wext")
        nc.sync.dma_start(out=wext_sb[:], in_=wext[:])

        for t in range(NT):
            rows = slice(t * P, (t + 1) * P)
            kt = ktot[t]

            idx_t = sb.tile([P, idx_cols[t]], I16, tag="idx")
            nc.sync.dma_start(
                out=idx_t[:], in_=idx[:, idx_off[t] : idx_off[t + 1]]
            )
            aux_t = sb.tile([P, 3], F32, tag="aux")
            nc.sync.dma_start(out=aux_t[:], in_=aux[rows, :])

            # Gather this tile's token rows: dma_gather per vocab window,
            # chunked to <=8 columns (1024 indices — the SWDGE ring limit).
            # Stream position i -> partition i%128 (the bag), column i//128.
            g_t = gp.tile([P, kt * EMBED], F32, tag="g")
            goff = 0
            ioff = 0
            for w in range(NWIN):
                rem = kmat[t][w]
                while rem > 0:
                    kw = min(rem, KCHUNK)
                    gv = g_t[:, goff * EMBED : (goff + kw) * EMBED].rearrange(
                        "p (k e) -> p k e", k=kw, e=EMBED
                    )
                    nc.gpsimd.dma_gather(
                        gv,
                        emb[w * WROWS :, :],
                        idx_t[:, ioff : ioff + 8 * kw],
                        P * kw,
                        P * kw,
                        EMBED,
                    )
                    goff += kw
                    ioff += 8 * kw
                    rem -= kw

            # Sum over the columns (strided view [P, e, kt], reduce innermost)
            s_t = sb.tile([P, EMBED], F32, tag="s")
            nc.vector.tensor_reduce(
                out=s_t[:],
                in_=g_t[:].rearrange("p (k e) -> p e k", k=kt, e=EMBED),
                axis=mybir.AxisListType.X,
                op=op.add,
            )

            # mean = sum * (1/max(len,1)); append the two select flags
            tr = sb.tile([P, EMBED + 2], F32, tag="tr")
            nc.vector.tensor_scalar_mul(
                out=tr[:, 0:EMBED], in0=s_t[:], scalar1=aux_t[:, 0:1]
            )
            nc.vector.tensor_copy(out=tr[:, EMBED : EMBED + 2], in_=aux_t[:, 1:3])

            # [P, 66] -> [66, P] so the projection contracts over E on partitions
            pT = ps.tile([EMBED + 2, P], F32, tag="pT", space="PSUM")
            nc.tensor.transpose(out=pT[:], in_=tr[:], identity=idt[:])
            mT = sb.tile([EMBED + 2, P], F32, tag="mT")
            nc.scalar.copy(out=mT[:], in_=pT[:])

            # out[128, 256] = meanT.T @ [W.T; b; null]: proj + bias + null select
            po = ps.tile([P, COND], F32, tag="po", space="PSUM")
            nc.tensor.matmul(out=po[:], lhsT=mT[:], rhs=wext_sb[:], start=True, stop=True)
            ob = sb.tile([P, COND], F32, tag="ob")
            nc.scalar.copy(out=ob[:], in_=po[:])
            nc.scalar.dma_start(out=out[rows, :], in_=ob[:])

    nc.compile()
    return nc


_CACHE: dict = {}


def _prep(token_ids, lengths):
    """Sort bags by length per core, split tokens by vocab window, compute
    column budgets. Returns (kmat, per-core idx arrays, per-core aux, perms)."""
    ids_all = np.asarray(token_ids).astype(np.int64, copy=False)
    lens_all = np.asarray(lengths).astype(np.int64, copy=False)

    cores = []
    for c in range(NCORES):
        ids = ids_all[c * BLOC : (c + 1) * BLOC]
        lens = lens_all[c * BLOC : (c + 1) * BLOC]
        order = np.argsort(-lens, kind="stable")
        ids, lens = ids[order], lens[order]
        new_ids = ids + ids // VROWS + 1  # remapped table row
        win = new_ids // WROWS  # 0..3
        rel = new_ids - win * WROWS  # 1..32767
        valid = np.arange(L)[None, :] < lens[:, None]
        aux = np.stack(
            [
                1.0 / np.maximum(lens, 1),
                (lens > 0),
                (lens == 0),
            ],
            axis=1,
        ).astype(np.float32)
        cores.append((order, lens, win, rel, valid, aux))

    # counts[c, t, w, b] = tokens of bag b (tile t, core c) in window w
    counts = np.zeros((NCORES, NT, NWIN, P), np.int64)
    for c, (_, _, win, _, valid, _) in enumerate(cores):
        for w in range(NWIN):
            cnt = ((win == w) & valid).sum(axis=1)  # [BLOC]
            counts[c, :, w, :] = cnt.reshape(NT, P)
    kmat = counts.max(axis=(0, 3))  # [NT, NWIN] shared across cores
    # ensure at least one column per tile so every tile has a gather
    for t in range(NT):
        if kmat[t].sum() == 0:
            kmat[t][0] = 1
    kmat = kmat.tolist()

    idx_arrs, aux_arrs, perms = [], [], []
    for c, (order, lens, win, rel, valid, aux) in enumerate(cores):
        blocks = []
        for t in range(NT):
            rows = slice(t * P, (t + 1) * P)
            winb, relb, validb = win[rows], rel[rows], valid[rows]
            for w in range(NWIN):
                kw = kmat[t][w]
                if kw == 0:
                    continue
                sel = (winb == w) & validb  # [P, L]
                cnt = sel.sum(axis=1)  # [P]
                # stable-partition each bag's window-w rel ids to the front
                pos = np.argsort(~sel, axis=1, kind="stable")[:, :kw]
                vals = np.take_along_axis(relb, pos, axis=1)
                colmask = np.arange(kw)[None, :] < cnt[:, None]
                padded = np.where(colmask, vals, 0)  # [P, kw]
                for c0 in range(0, kw, KCHUNK):
                    chunk = padded[:, c0 : c0 + KCHUNK]
                    flat = chunk.T.ravel()  # stream order i = col*128 + bag
                    blk = flat.reshape(-1, 16).T  # [16, P*kc/16]
                    blocks.append(np.tile(blk, (8, 1)))
        idx_arrs.append(
            np.ascontiguousarray(np.concatenate(blocks, axis=1).astype(np.int16))
        )
        aux_arrs.append(aux)
        perms.append(order)
    return kmat, idx_arrs, aux_arrs, perms


def make_in_maps(token_ids, lengths, emb_table, W, b, null_emb):
    kmat, idx_arrs, aux_arrs, perms = _prep(token_ids, lengths)

    emb_src = np.asarray(emb_table, dtype=np.float32)
    emb_ext = np.zeros((TROWS, EMBED), np.float32)
    new_rows = np.arange(VOCAB) + np.arange(VOCAB) // VROWS + 1
    emb_ext[new_rows] = emb_src

    wext = np.concatenate(
        [
            np.asarray(W, dtype=np.float32).T,  # [64, 256]
            np.asarray(b, dtype=np.float32)[None, :],
            np.asarray(null_emb, dtype=np.float32)[None, :],
        ]
    )  # [66, 256]
    in_maps = [
        {
            "idx": idx_arrs[c],
            "aux": aux_arrs[c],
            "emb": emb_ext,
            "wext": wext,
        }
        for c in range(NCORES)
    ]
    return kmat, in_maps, perms


def kernel(token_ids, lengths, emb_table, W, b, null_emb, **run_kwargs):
    from concourse.bass_utils import run_bass_kernel_spmd

    kmat, in_maps, perms = make_in_maps(
        token_ids, lengths, emb_table, W, b, null_emb
    )
    key = tuple(tuple(kr) for kr in kmat)
    if key not in _CACHE:
        _CACHE[key] = build_nc(kmat)
    nc = _CACHE[key]
    res = run_bass_kernel_spmd(nc, in_maps, core_ids=list(range(NCORES)), **run_kwargs)
    global _LAST_RES
    _LAST_RES = res
    out = np.empty((B, COND), np.float32)
    for c in range(NCORES):
        out[c * BLOC + perms[c]] = res.results[c]["out"]
    return out
